# revision 9
# baseline (speedup 1.0000x reference)
"""TRN2 Bass kernel for nn_Architecture_979252544349 (dense_transformer).

Data-parallel over batch: 8 NeuronCores, one batch element each, no collectives.

Layout strategy:
  - activations transposed [d, L] for matmul contraction; host pre-transposes
    inputs; mid-network re-transposes via DMA-transpose through DRAM scratch
  - everything on-chip in [128, 1024] bf16 "granules" under one FIFO-planned
    pool tag per block
  - attention per (head, 128-query tile):
      scores(psum) -> +mask(-1e9 diag) -> p=exp(s*x) -> cum=prefix_scan(p)
      -> dist2=(cum-tot)*(-pos) -> dist=exp(.5*ln(dist2/tot))
      -> eff=exp(gamma*dist) -> w=max(eff,1e-5)*scores -> a=exp(s*w)/rowsum
      -> PE-transpose(a) -> outT += V.T @ aT
  - ACT functions restricted to the natural_log_exp_and_others table set
    (Exp, Ln, Identity, Relu, Copy): zero table switches
  - matmuls bf16 with f32 psum accumulation; elementwise f32
"""
import numpy as np
import ml_dtypes

import concourse.bass as bass
from concourse import bacc, mybir, bass_utils
from concourse.tile import TileContext

BF16 = mybir.dt.bfloat16
F32 = mybir.dt.float32
AF = mybir.ActivationFunctionType
OP = mybir.AluOpType
NPBF = ml_dtypes.bfloat16

B, L, D, H, DFF = 8, 1024, 1024, 8, 4096
NEG_MASK = -1e9
EPS_LN = 1e-5
GR = 1024  # granule free size



# Pin all ACT table choices to natural_log_exp_and_others: every function this
# kernel uses (Exp, Ln, Identity, Relu, Copy) is in that one set, but the
# default chooser flip-flops between sets (hundreds of ~2.7us table loads).
from concourse import hw_specs as _hw_specs
from concourse import bacc as _bacc_mod
import functools as _ft

_orig_get_tables = _hw_specs.get_activation_tables.__wrapped__ if hasattr(_hw_specs.get_activation_tables, "__wrapped__") else _hw_specs.get_activation_tables

@_ft.cache
def _pinned_tables(module_arch):
    tabs = _orig_get_tables(module_arch)
    keep = "natural_log_exp_and_others"
    return {k: (v if k == keep else set()) for k, v in tabs.items()}

_hw_specs.get_activation_tables = _pinned_tables
_bacc_mod.get_activation_tables = _pinned_tables

# ================================================================ host prep

def prep_stationary(W):
    """W [dout, din] -> [dout//128, 128, din] bf16; A[do,p,di*128+c]=W[do*128+c,di*128+p]."""
    dout, din = W.shape
    ndo, ndi = dout // 128, din // 128
    A = W.reshape(ndo, 128, ndi, 128).transpose(0, 3, 2, 1).reshape(ndo, 128, din)
    return np.ascontiguousarray(A).astype(NPBF)


def prep_moving(W):
    """W [dout, din] -> W.T tiled [din//128, 128, dout] bf16."""
    dout, din = W.shape
    A = np.ascontiguousarray(W.T).reshape(din // 128, 128, dout)
    return A.astype(NPBF)


def prep_bias_col(b):
    return np.ascontiguousarray(np.asarray(b, np.float32).reshape(-1, 128, 1))


def prep_bias_row(b):
    return np.ascontiguousarray(np.asarray(b).reshape(1, -1)).astype(NPBF)


# ================================================================ builder

class Blk:
    def __init__(self, name, idx, d, vlen, strict, ffn):
        self.name, self.idx = name, idx
        self.d, self.vlen, self.strict, self.ffn = d, vlen, strict, ffn
        self.dk, self.dv = d // H, vlen // H


BLOCKS = {
    "y": Blk("y", 0, 2 * D, 2 * D, strict=False, ffn=True),
    "x": Blk("x", 1, D, D, strict=False, ffn=False),
    "h": Blk("h", 2, D, 2 * D, strict=True, ffn=True),
}


def build_kernel():
    nc = bacc.Bacc("TRN2")
    io = {}

    def din(name, shape, dt=BF16):
        io[name] = nc.dram_tensor(name, shape, dt, kind="ExternalInput")
        return io[name]

    din("qaT", [16, 128, L]); din("qa_n", [8, 128, 2 * D])
    din("qT", [8, 128, L]); din("q_n", [8, 128, D])
    din("negpos", [128, 2048], F32)
    din("mask_incl", [128, 128], F32); din("mask_strict", [128, 128], F32)
    din("identity", [128, 128]); din("ones_row", [1, 128])
    din("gammas", [128, 3 * H], F32)
    out = nc.dram_tensor("out", [8, 128, D], F32, kind="ExternalOutput")

    for b in BLOCKS.values():
        d, vlen, n = b.d, b.vlen, b.name
        din(f"{n}_Wk_st", [d // 128, 128, d]); din(f"{n}_bk", [d // 128, 128, 1], F32)
        din(f"{n}_Wv_mv", [vlen // 128, 128, vlen]); din(f"{n}_bv_row", [1, vlen])
        din(f"{n}_Wo_mv", [vlen // 128, 128, d]); din(f"{n}_bo_row", [1, d])
        if b.ffn:
            din(f"{n}_W1_st", [DFF // 128, 128, d]); din(f"{n}_b1", [DFF // 128, 128, 1], F32)
            din(f"{n}_W2_mv", [DFF // 128, 128, d]); din(f"{n}_b2_row", [1, d])

    with TileContext(nc) as tc:
        with tc.tile_pool(name="dscr", bufs=1, space="DRAM") as dp, \
             tc.tile_pool(name="consts", bufs=1) as cpool:
            y_dram = dp.tile([L, 2 * D], BF16, tag="y_scr")
            x_dram = dp.tile([L, D], BF16, tag="x_scr")
            qln_y = dp.tile([L, 2 * D], BF16, tag="qln_y")
            qln_h = dp.tile([L, D], BF16, tag="qln_h")

            C = {}
            for nm in ["negpos", "mask_incl", "mask_strict", "identity",
                       "ones_row", "gammas"]:
                t = cpool.tile(list(io[nm].shape), io[nm].dtype, tag=nm)
                nc.sync.dma_start(t[:], io[nm][:])
                C[nm] = t
            zeros = cpool.tile([128, L], F32, tag="zeros")
            nc.vector.memset(zeros[:], 0.0)
            C["zeros"] = zeros

            run_block(nc, tc, BLOCKS["y"], io, C, src_T=io["qaT"], src_n=io["qa_n"],
                      out_n_dram=y_dram, qln_dram=qln_y, out_io=None)
            run_block(nc, tc, BLOCKS["x"], io, C, src_T=io["qT"], src_n=io["q_n"],
                      out_n_dram=x_dram, qln_dram=None, out_io=None)
            run_block(nc, tc, BLOCKS["h"], io, C, src_T=x_dram, src_n=x_dram,
                      out_n_dram=None, qln_dram=qln_h, out_io=out,
                      vsrc_dram=y_dram, h_mode=True)

    return nc, io


def layernorm_grans(nc, pool, grans, cols, out_aps):
    """LayerNorm along the feature axis spread over `grans` (each [128, cols]).
    gains=1, biases=0. out_aps: list of APs matching grans."""
    ng = len(grans)
    nch = cols // 512
    st = pool.tile([128, 6 * nch * ng], F32, tag="ln_st")
    for g, r in enumerate(grans):
        for c in range(nch):
            k = g * nch + c
            nc.vector.bn_stats(st[:, k * 6:(k + 1) * 6], r[:, c * 512:(c + 1) * 512])
    mv = pool.tile([128, 2], F32, tag="ln_mv")
    nc.vector.bn_aggr(mv[:], st[:].rearrange("p (c s) -> p c s", c=nch * ng))
    rstd = pool.tile([128, 1], F32, tag="ln_rstd")
    nc.vector.tensor_scalar_add(rstd[:], mv[:, 1:2], EPS_LN)
    nc.scalar.activation(rstd[:], rstd[:], AF.Ln)
    nc.scalar.activation(rstd[:], rstd[:], AF.Exp, scale=-0.5)
    nmr = pool.tile([128, 1], F32, tag="ln_nmr")
    nc.vector.tensor_scalar_mul(nmr[:], mv[:, 0:1], -1.0)
    nc.vector.tensor_mul(nmr[:], nmr[:], rstd[:])
    for g, r in enumerate(grans):
        nc.scalar.activation(out_aps[g], r, AF.Identity, bias=nmr[:], scale=rstd[:])


def run_block(nc, tc, blk, io, C, src_T, src_n, out_n_dram, qln_dram,
              out_io=None, vsrc_dram=None, h_mode=False):
    n, d, vlen, dk, dv = blk.name, blk.d, blk.vlen, blk.dk, blk.dv
    nd, nv, nlt = d // 128, vlen // 128, L // 128
    ngr_d = d // GR      # granules per feature row (1 or 2)
    ngr_v = vlen // GR
    inv_s = 1.0 / float(np.sqrt(dk))
    mask = C["mask_strict"] if blk.strict else C["mask_incl"]
    nbufs = {"y": 64, "x": 40, "h": 56}[n]

    with tc.tile_pool(name=f"{n}_pool", bufs=nbufs) as gp:
        g_tile = lambda: gp.tile([128, GR], BF16, tag="G", name="G")

        # ---- 1. XT granules (+ value-source transposed for h) ----
        XT = []
        for di in range(nd):
            t = g_tile()
            if h_mode:
                nc.sync.dma_start_transpose(t[:], src_T[:, di * 128:(di + 1) * 128])
            else:
                nc.sync.dma_start(t[:], src_T[di])
            XT.append(t)
        if h_mode:
            VT_src = []
            for di in range(nv):
                t = g_tile()
                nc.sync.dma_start_transpose(t[:], vsrc_dram[:, di * 128:(di + 1) * 128])
                VT_src.append(t)
        else:
            VT_src = XT

        # ---- 2. QKT (transposed linear) ----
        QKT = []
        with tc.tile_pool(name=f"{n}_wk", bufs=2) as wp, \
             tc.tile_pool(name=f"{n}_pk", bufs=4, space="PSUM") as pp:
            for do in range(nd):
                w = wp.tile([128, d], BF16, tag="wst")
                nc.sync.dma_start(w[:], io[f"{n}_Wk_st"][do])
                bt = wp.tile([128, 1], F32, tag="bst")
                nc.sync.dma_start(bt[:], io[f"{n}_bk"][do])
                o = g_tile()
                for lc in range(L // 512):
                    ps = pp.tile([128, 512], F32, tag="ps")
                    for di in range(nd):
                        nc.tensor.matmul(ps[:], w[:, di * 128:(di + 1) * 128],
                                         XT[di][:, lc * 512:(lc + 1) * 512],
                                         start=(di == 0), stop=(di == nd - 1))
                    nc.scalar.activation(o[:, lc * 512:(lc + 1) * 512], ps[:],
                                         AF.Identity, bias=bt[:], scale=1.0)
                QKT.append(o)

        # ---- 3. V natural: V[lt] = list of ngr_v granules [128, GR] ----
        V = [[g_tile() for _ in range(ngr_v)] for _ in range(nlt)]
        with tc.tile_pool(name=f"{n}_wv", bufs=nv + 2) as wp, \
             tc.tile_pool(name=f"{n}_pv", bufs=4, space="PSUM") as pp:
            brow = wp.tile([1, vlen], BF16, tag="bvrow", bufs=1)
            nc.sync.dma_start(brow[:], io[f"{n}_bv_row"][:])
            for vc in range(vlen // 512):
                wt = []
                for di in range(nv):
                    w = wp.tile([128, 512], BF16, tag="wmv", name="wmv")
                    nc.sync.dma_start(w[:], io[f"{n}_Wv_mv"][di][:, vc * 512:(vc + 1) * 512])
                    wt.append(w)
                for lt in range(nlt):
                    ps = pp.tile([128, 512], F32, tag="ps")
                    for di in range(nv):
                        nc.tensor.matmul(ps[:], VT_src[di][:, lt * 128:(lt + 1) * 128],
                                         wt[di][:], start=(di == 0), stop=False)
                    nc.tensor.matmul(ps[:], C["ones_row"][:],
                                     brow[:, vc * 512:(vc + 1) * 512],
                                     start=False, stop=True)
                    g, col = (vc * 512) // GR, (vc * 512) % GR
                    nc.scalar.activation(V[lt][g][:, col:col + 512], ps[:], AF.Copy)

        # ---- 4. attention ----
        outT = [g_tile() for _ in range(nv)]
        ndk = dk // 128
        with tc.tile_pool(name=f"{n}_ws", bufs=2) as ws, \
             tc.tile_pool(name=f"{n}_wsT", bufs=12) as wsT, \
             tc.tile_pool(name=f"{n}_pa", bufs=2, space="PSUM") as pa, \
             tc.tile_pool(name=f"{n}_po", bufs=3, space="PSUM") as po:
            for h in range(H):
                qh = [QKT[(h * dk) // 128 + i] for i in range(ndk)]
                gam = C["gammas"][:, blk.idx * H + h:blk.idx * H + h + 1]
                for t in range(nlt):
                    Lk = (t + 1) * 128
                    ps_s = pa.tile([128, 1024], F32, tag="ps_s")
                    for kc in range((Lk + 511) // 512):
                        nk = min(512, Lk - kc * 512)
                        for ki in range(ndk):
                            nc.tensor.matmul(
                                ps_s[:, kc * 512:kc * 512 + nk],
                                qh[ki][:, t * 128:(t + 1) * 128],
                                qh[ki][:, kc * 512:kc * 512 + nk],
                                start=(ki == 0), stop=(ki == ndk - 1))
                    nc.vector.tensor_add(ps_s[:, t * 128:Lk], ps_s[:, t * 128:Lk], mask[:])
                    p = ws.tile([128, L], F32, tag="p")
                    nc.scalar.activation(p[:, :Lk], ps_s[:, :Lk], AF.Exp, scale=inv_s)
                    cum = ws.tile([128, L], F32, tag="cum")
                    nc.vector.tensor_tensor_scan(cum[:, :Lk], p[:, :Lk],
                                                 C["zeros"][:, :Lk], 0.0,
                                                 op0=OP.add, op1=OP.add)
                    sig = cum[:, Lk - 1:Lk]
                    rcp = ws.tile([128, 1], F32, tag="rcp")
                    nc.vector.tensor_scalar_add(rcp[:], sig, 1e-30)
                    nc.vector.reciprocal(rcp[:], rcp[:])
                    d2 = ws.tile([128, L], F32, tag="d2")
                    np_sl = C["negpos"][:, 1024 - 128 * t:1024 - 128 * t + Lk]
                    nc.vector.scalar_tensor_tensor(d2[:, :Lk], cum[:, :Lk], sig,
                                                   np_sl, op0=OP.subtract, op1=OP.mult)
                    nc.scalar.activation(d2[:, :Lk], d2[:, :Lk], AF.Ln, scale=rcp[:])
                    nc.scalar.activation(d2[:, :Lk], d2[:, :Lk], AF.Exp, scale=0.5)
                    nc.scalar.activation(d2[:, :Lk], d2[:, :Lk], AF.Exp, scale=gam)
                    wv = ws.tile([128, L], F32, tag="wv")
                    nc.vector.scalar_tensor_tensor(wv[:, :Lk], d2[:, :Lk], 1e-5,
                                                   ps_s[:, :Lk], op0=OP.max, op1=OP.mult)
                    au = ws.tile([128, L], F32, tag="au")
                    sig2 = ws.tile([128, 1], F32, tag="sig2")
                    nc.scalar.activation(au[:, :Lk], wv[:, :Lk], AF.Exp, scale=inv_s,
                                         accum_out=sig2[:])
                    rcp2 = ws.tile([128, 1], F32, tag="rcp2")
                    nc.vector.tensor_scalar_add(rcp2[:], sig2[:], 1e-30)
                    nc.vector.reciprocal(rcp2[:], rcp2[:])
                    abf = ws.tile([128, L], BF16, tag="abf")
                    nc.vector.tensor_scalar_mul(abf[:, :Lk], au[:, :Lk], rcp2[:])
                    aT = []
                    for s in range(t + 1):
                        a = wsT.tile([128, 128], BF16, tag="aT")
                        nc.sync.dma_start_transpose(a[:], abf[:, s * 128:(s + 1) * 128])
                        aT.append(a)
                    for dvi in range(dv // 128):
                        c = h * dv + dvi * 128
                        g, col = c // GR, c % GR
                        ps_o = po.tile([128, 128], F32, tag="ps_o")
                        for s in range(t + 1):
                            nc.tensor.matmul(ps_o[:], V[s][g][:, col:col + 128],
                                             aT[s][:], start=(s == 0), stop=(s == t))
                        ot = outT[c // 128]
                        nc.vector.tensor_copy(ot[:, t * 128:(t + 1) * 128], ps_o[:])

        # ---- 5. a2 natural + residual -> R granules; LN1 -> QLN ----
        R = [[None] * ngr_d for _ in range(nlt)]
        with tc.tile_pool(name=f"{n}_wo", bufs=2 * nv + 2) as wp, \
             tc.tile_pool(name=f"{n}_res", bufs=3) as rp, \
             tc.tile_pool(name=f"{n}_po2", bufs=4, space="PSUM") as pp:
            brow = wp.tile([1, d], BF16, tag="borow", bufs=1)
            nc.sync.dma_start(brow[:], io[f"{n}_bo_row"][:])
            for lt in range(nlt):
                for g in range(ngr_d):
                    R[lt][g] = g_tile()
            for dc in range(d // 512):
                wt = []
                for di in range(nv):
                    w = wp.tile([128, 512], BF16, tag="womv")
                    nc.sync.dma_start(w[:], io[f"{n}_Wo_mv"][di][:, dc * 512:(dc + 1) * 512])
                    wt.append(w)
                for lt in range(nlt):
                    ps = pp.tile([128, 512], F32, tag="ps")
                    for di in range(nv):
                        nc.tensor.matmul(ps[:], outT[di][:, lt * 128:(lt + 1) * 128],
                                         wt[di][:], start=(di == 0), stop=False)
                    nc.tensor.matmul(ps[:], C["ones_row"][:],
                                     brow[:, dc * 512:(dc + 1) * 512],
                                     start=False, stop=True)
                    xn = rp.tile([128, 512], BF16, tag="xn")
                    if h_mode:
                        nc.sync.dma_start(
                            xn[:], src_n[lt * 128:(lt + 1) * 128, dc * 512:(dc + 1) * 512])
                    else:
                        nc.sync.dma_start(xn[:], src_n[lt][:, dc * 512:(dc + 1) * 512])
                    g, col = (dc * 512) // GR, (dc * 512) % GR
                    nc.vector.tensor_add(R[lt][g][:, col:col + 512], ps[:], xn[:])

        QLN = [[None] * ngr_d for _ in range(nlt)]
        with tc.tile_pool(name=f"{n}_ln", bufs=3) as lp:
            for lt in range(nlt):
                for g in range(ngr_d):
                    QLN[lt][g] = g_tile()
                layernorm_grans(nc, lp, [R[lt][g][:] for g in range(ngr_d)], GR,
                                [QLN[lt][g][:] for g in range(ngr_d)])

        if not blk.ffn:
            for lt in range(nlt):
                for g in range(ngr_d):
                    nc.sync.dma_start(
                        out_n_dram[lt * 128:(lt + 1) * 128, g * GR:(g + 1) * GR],
                        QLN[lt][g][:])
            return

        # ---- 6. FFN ----
        for lt in range(nlt):
            for g in range(ngr_d):
                nc.sync.dma_start(
                    qln_dram[lt * 128:(lt + 1) * 128, g * GR:(g + 1) * GR],
                    QLN[lt][g][:])
        QLT = []
        for di in range(nd):
            t = g_tile()
            nc.sync.dma_start_transpose(t[:], qln_dram[:, di * 128:(di + 1) * 128])
            QLT.append(t)

        F1 = []
        with tc.tile_pool(name=f"{n}_w1", bufs=2) as wp, \
             tc.tile_pool(name=f"{n}_p1", bufs=4, space="PSUM") as pp:
            for do in range(DFF // 128):
                w = wp.tile([128, d], BF16, tag="w1st")
                nc.sync.dma_start(w[:], io[f"{n}_W1_st"][do])
                bt = wp.tile([128, 1], F32, tag="b1st")
                nc.sync.dma_start(bt[:], io[f"{n}_b1"][do])
                o = g_tile()
                for lc in range(L // 512):
                    ps = pp.tile([128, 512], F32, tag="ps")
                    for di in range(nd):
                        nc.tensor.matmul(ps[:], w[:, di * 128:(di + 1) * 128],
                                         QLT[di][:, lc * 512:(lc + 1) * 512],
                                         start=(di == 0), stop=(di == nd - 1))
                    nc.scalar.activation(o[:, lc * 512:(lc + 1) * 512], ps[:],
                                         AF.Relu, bias=bt[:], scale=1.0)
                F1.append(o)

        # FFN2 natural, residual accumulated in place into QLN, then LN2
        with tc.tile_pool(name=f"{n}_w2", bufs=DFF // 128 + 4) as wp, \
             tc.tile_pool(name=f"{n}_ln2", bufs=3) as lp, \
             tc.tile_pool(name=f"{n}_p2", bufs=4, space="PSUM") as pp:
            brow = wp.tile([1, d], BF16, tag="b2row", bufs=1)
            nc.sync.dma_start(brow[:], io[f"{n}_b2_row"][:])
            for dc in range(d // 512):
                wt = []
                for di in range(DFF // 128):
                    w = wp.tile([128, 512], BF16, tag="w2mv")
                    nc.sync.dma_start(w[:], io[f"{n}_W2_mv"][di][:, dc * 512:(dc + 1) * 512])
                    wt.append(w)
                for lt in range(nlt):
                    ps = pp.tile([128, 512], F32, tag="ps")
                    for di in range(DFF // 128):
                        nc.tensor.matmul(ps[:], F1[di][:, lt * 128:(lt + 1) * 128],
                                         wt[di][:], start=(di == 0), stop=False)
                    nc.tensor.matmul(ps[:], C["ones_row"][:],
                                     brow[:, dc * 512:(dc + 1) * 512],
                                     start=False, stop=True)
                    g, col = (dc * 512) // GR, (dc * 512) % GR
                    # in-place residual: QLN <- f2 + QLN
                    nc.vector.tensor_add(QLN[lt][g][:, col:col + 512], ps[:],
                                         QLN[lt][g][:, col:col + 512])
            for lt in range(nlt):
                if out_io is not None:  # final block: f32 natural straight out
                    o = lp.tile([128, d], F32, tag="ofin")
                    layernorm_grans(nc, lp, [QLN[lt][g][:] for g in range(ngr_d)], GR,
                                    [o[:, g * GR:(g + 1) * GR] for g in range(ngr_d)])
                    nc.sync.dma_start(out_io[lt], o[:])
                else:
                    obf = [g_tile() for _ in range(ngr_d)]
                    layernorm_grans(nc, lp, [QLN[lt][g][:] for g in range(ngr_d)], GR,
                                    [obf[g][:] for g in range(ngr_d)])
                    for g in range(ngr_d):
                        nc.sync.dma_start(
                            out_n_dram[lt * 128:(lt + 1) * 128, g * GR:(g + 1) * GR],
                            obf[g][:])


# ================================================================ runner

_CACHE = {}


def _get_compiled():
    if "nc" not in _CACHE:
        nc, io = build_kernel()
        nc.finalize()
        _CACHE["nc"] = nc
        _CACHE["io"] = io
    return _CACHE["nc"], _CACHE["io"]


def _softplus(x):
    return np.logaddexp(0.0, x)


def prep_in_maps(q_embed_data, qa_embed_data, params):
    q = np.asarray(q_embed_data, np.float32)
    qa = np.asarray(qa_embed_data, np.float32)

    shared = {}
    p = np.arange(128)[:, None]
    u = np.arange(2048)[None, :]
    shared["negpos"] = (-np.abs(p + 1024.0 - u)).astype(np.float32)
    i_ = np.arange(128)[:, None]
    j_ = np.arange(128)[None, :]
    shared["mask_incl"] = np.where(j_ <= i_, 0.0, NEG_MASK).astype(np.float32)
    shared["mask_strict"] = np.where(j_ < i_, 0.0, NEG_MASK).astype(np.float32)
    shared["identity"] = np.eye(128, dtype=NPBF)
    shared["ones_row"] = np.ones((1, 128), dtype=NPBF)
    gam = np.zeros((128, 3 * H), np.float32)
    for nm, bidx in [("y", 0), ("x", 1), ("h", 2)]:
        gvals = -_softplus(np.asarray(params[nm]["gam"], np.float32).reshape(H))
        gam[:, bidx * H:(bidx + 1) * H] = gvals[None, :]
    shared["gammas"] = gam

    for nm in ["y", "x", "h"]:
        bp = params[nm]
        shared[f"{nm}_Wk_st"] = prep_stationary(np.asarray(bp["Wk"], np.float32))
        shared[f"{nm}_bk"] = prep_bias_col(bp["bk"])
        shared[f"{nm}_Wv_mv"] = prep_moving(np.asarray(bp["Wv"], np.float32))
        shared[f"{nm}_bv_row"] = prep_bias_row(bp["bv"])
        shared[f"{nm}_Wo_mv"] = prep_moving(np.asarray(bp["Wo"], np.float32))
        shared[f"{nm}_bo_row"] = prep_bias_row(bp["bo"])
        if "W1" in bp:
            shared[f"{nm}_W1_st"] = prep_stationary(np.asarray(bp["W1"], np.float32))
            shared[f"{nm}_b1"] = prep_bias_col(bp["b1"])
            shared[f"{nm}_W2_mv"] = prep_moving(np.asarray(bp["W2"], np.float32))
            shared[f"{nm}_b2_row"] = prep_bias_row(bp["b2"])

    in_maps = []
    for b in range(B):
        m = dict(shared)
        m["qaT"] = np.ascontiguousarray(qa[b].T).reshape(16, 128, L).astype(NPBF)
        m["qa_n"] = np.ascontiguousarray(qa[b]).reshape(8, 128, 2 * D).astype(NPBF)
        m["qT"] = np.ascontiguousarray(q[b].T).reshape(8, 128, L).astype(NPBF)
        m["q_n"] = np.ascontiguousarray(q[b]).reshape(8, 128, D).astype(NPBF)
        in_maps.append(m)
    return in_maps


def kernel(q_embed_data, qa_embed_data, params):
    nc, io = _get_compiled()
    in_maps = prep_in_maps(q_embed_data, qa_embed_data, params)
    res = bass_utils.run_bass_kernel_spmd(nc, in_maps, core_ids=list(range(B)))
    outs = [r["out"].reshape(L, D) for r in res.results]
    return np.stack(outs, 0).astype(np.float32)


# revision 10
# speedup vs baseline: 1.3508x; 1.3508x over previous
"""TRN2 Bass kernel for nn_Architecture_979252544349 (dense_transformer).

Data-parallel over batch: 8 NeuronCores, one batch element each, no collectives.

Layout strategy:
  - activations transposed [d, L] for matmul contraction; host pre-transposes
    inputs; mid-network re-transposes via DMA-transpose through DRAM scratch
  - everything on-chip in [128, 1024] bf16 "granules" under one FIFO-planned
    pool tag per block
  - attention per (head, 128-query tile):
      scores(psum) -> +mask(-1e9 diag) -> p=exp(s*x) -> cum=prefix_scan(p)
      -> dist2=(cum-tot)*(-pos) -> dist=exp(.5*ln(dist2/tot))
      -> eff=exp(gamma*dist) -> w=max(eff,1e-5)*scores -> a=exp(s*w)/rowsum
      -> PE-transpose(a) -> outT += V.T @ aT
  - ACT functions restricted to the natural_log_exp_and_others table set
    (Exp, Ln, Identity, Relu, Copy): zero table switches
  - matmuls bf16 with f32 psum accumulation; elementwise f32
"""
import numpy as np
import ml_dtypes

import concourse.bass as bass
from concourse import bacc, mybir, bass_utils
from concourse.tile import TileContext

BF16 = mybir.dt.bfloat16
F32 = mybir.dt.float32
AF = mybir.ActivationFunctionType
OP = mybir.AluOpType
NPBF = ml_dtypes.bfloat16

B, L, D, H, DFF = 8, 1024, 1024, 8, 4096
NEG_MASK = -1e9
EPS_LN = 1e-5
GR = 1024  # granule free size



# Pin all ACT table choices to natural_log_exp_and_others: every function this
# kernel uses (Exp, Ln, Identity, Relu, Copy) is in that one set, but the
# default chooser flip-flops between sets (hundreds of ~2.7us table loads).
from concourse import hw_specs as _hw_specs
from concourse import bacc as _bacc_mod
import functools as _ft

_orig_get_tables = _hw_specs.get_activation_tables.__wrapped__ if hasattr(_hw_specs.get_activation_tables, "__wrapped__") else _hw_specs.get_activation_tables

@_ft.cache
def _pinned_tables(module_arch):
    tabs = _orig_get_tables(module_arch)
    keep = "natural_log_exp_and_others"
    return {k: (v if k == keep else set()) for k, v in tabs.items()}

_hw_specs.get_activation_tables = _pinned_tables
_bacc_mod.get_activation_tables = _pinned_tables

# ================================================================ host prep

def prep_stationary(W):
    """W [dout, din] -> [dout//128, 128, din] bf16; A[do,p,di*128+c]=W[do*128+c,di*128+p]."""
    dout, din = W.shape
    ndo, ndi = dout // 128, din // 128
    A = W.reshape(ndo, 128, ndi, 128).transpose(0, 3, 2, 1).reshape(ndo, 128, din)
    return np.ascontiguousarray(A).astype(NPBF)


def prep_moving(W):
    """W [dout, din] -> W.T tiled [din//128, 128, dout] bf16."""
    dout, din = W.shape
    A = np.ascontiguousarray(W.T).reshape(din // 128, 128, dout)
    return A.astype(NPBF)


def prep_bias_col(b):
    return np.ascontiguousarray(np.asarray(b, np.float32).reshape(-1, 128, 1))


def prep_bias_row(b):
    return np.ascontiguousarray(np.asarray(b).reshape(1, -1)).astype(NPBF)


# ================================================================ builder

class Blk:
    def __init__(self, name, idx, d, vlen, strict, ffn):
        self.name, self.idx = name, idx
        self.d, self.vlen, self.strict, self.ffn = d, vlen, strict, ffn
        self.dk, self.dv = d // H, vlen // H


BLOCKS = {
    "y": Blk("y", 0, 2 * D, 2 * D, strict=False, ffn=True),
    "x": Blk("x", 1, D, D, strict=False, ffn=False),
    "h": Blk("h", 2, D, 2 * D, strict=True, ffn=True),
}


def build_kernel():
    nc = bacc.Bacc("TRN2")
    io = {}

    def din(name, shape, dt=BF16):
        io[name] = nc.dram_tensor(name, shape, dt, kind="ExternalInput")
        return io[name]

    din("qaT", [16, 128, L]); din("qa_n", [8, 128, 2 * D])
    din("qT", [8, 128, L]); din("q_n", [8, 128, D])
    din("negpos", [128, 2048], F32)
    din("mask_incl", [128, 128], F32); din("mask_strict", [128, 128], F32)
    din("identity", [128, 128]); din("ones_row", [1, 128])
    din("gammas", [128, 3 * H], F32)
    out = nc.dram_tensor("out", [8, 128, D], F32, kind="ExternalOutput")

    for b in BLOCKS.values():
        d, vlen, n = b.d, b.vlen, b.name
        din(f"{n}_Wk_st", [d // 128, 128, d]); din(f"{n}_bk", [d // 128, 128, 1], F32)
        din(f"{n}_Wv_mv", [vlen // 128, 128, vlen]); din(f"{n}_bv_row", [1, vlen])
        din(f"{n}_Wo_mv", [vlen // 128, 128, d]); din(f"{n}_bo_row", [1, d])
        if b.ffn:
            din(f"{n}_W1_st", [DFF // 128, 128, d]); din(f"{n}_b1", [DFF // 128, 128, 1], F32)
            din(f"{n}_W2_mv", [DFF // 128, 128, d]); din(f"{n}_b2_row", [1, d])

    with TileContext(nc) as tc:
        with tc.tile_pool(name="dscr", bufs=1, space="DRAM") as dp, \
             tc.tile_pool(name="consts", bufs=1) as cpool:
            y_dram = dp.tile([L, 2 * D], BF16, tag="y_scr")
            x_dram = dp.tile([L, D], BF16, tag="x_scr")
            qln_y = dp.tile([L, 2 * D], BF16, tag="qln_y")
            qln_h = dp.tile([L, D], BF16, tag="qln_h")

            C = {}
            for nm in ["negpos", "mask_incl", "mask_strict", "identity",
                       "ones_row", "gammas"]:
                t = cpool.tile(list(io[nm].shape), io[nm].dtype, tag=nm)
                nc.sync.dma_start(t[:], io[nm][:])
                C[nm] = t
            zeros = cpool.tile([128, L], F32, tag="zeros")
            nc.vector.memset(zeros[:], 0.0)
            C["zeros"] = zeros

            run_block(nc, tc, BLOCKS["y"], io, C, src_T=io["qaT"], src_n=io["qa_n"],
                      out_n_dram=y_dram, qln_dram=qln_y, out_io=None)
            run_block(nc, tc, BLOCKS["x"], io, C, src_T=io["qT"], src_n=io["q_n"],
                      out_n_dram=x_dram, qln_dram=None, out_io=None)
            run_block(nc, tc, BLOCKS["h"], io, C, src_T=x_dram, src_n=x_dram,
                      out_n_dram=None, qln_dram=qln_h, out_io=out,
                      vsrc_dram=y_dram, h_mode=True)

    return nc, io


def layernorm_grans(nc, pool, grans, cols, out_aps):
    """LayerNorm along the feature axis spread over `grans` (each [128, cols]).
    gains=1, biases=0. out_aps: list of APs matching grans."""
    ng = len(grans)
    nch = cols // 512
    st = pool.tile([128, 6 * nch * ng], F32, tag="ln_st")
    for g, r in enumerate(grans):
        for c in range(nch):
            k = g * nch + c
            nc.vector.bn_stats(st[:, k * 6:(k + 1) * 6], r[:, c * 512:(c + 1) * 512])
    mv = pool.tile([128, 2], F32, tag="ln_mv")
    nc.vector.bn_aggr(mv[:], st[:].rearrange("p (c s) -> p c s", c=nch * ng))
    rstd = pool.tile([128, 1], F32, tag="ln_rstd")
    nc.vector.tensor_scalar_add(rstd[:], mv[:, 1:2], EPS_LN)
    nc.scalar.activation(rstd[:], rstd[:], AF.Ln)
    nc.scalar.activation(rstd[:], rstd[:], AF.Exp, scale=-0.5)
    nmr = pool.tile([128, 1], F32, tag="ln_nmr")
    nc.vector.tensor_scalar_mul(nmr[:], mv[:, 0:1], -1.0)
    nc.vector.tensor_mul(nmr[:], nmr[:], rstd[:])
    for g, r in enumerate(grans):
        nc.scalar.activation(out_aps[g], r, AF.Identity, bias=nmr[:], scale=rstd[:])


def run_block(nc, tc, blk, io, C, src_T, src_n, out_n_dram, qln_dram,
              out_io=None, vsrc_dram=None, h_mode=False):
    n, d, vlen, dk, dv = blk.name, blk.d, blk.vlen, blk.dk, blk.dv
    nd, nv, nlt = d // 128, vlen // 128, L // 128
    ngr_d = d // GR      # granules per feature row (1 or 2)
    ngr_v = vlen // GR
    inv_s = 1.0 / float(np.sqrt(dk))
    mask = C["mask_strict"] if blk.strict else C["mask_incl"]
    nbufs = {"y": 64, "x": 40, "h": 56}[n]

    with tc.tile_pool(name=f"{n}_pool", bufs=nbufs) as gp:
        g_tile = lambda: gp.tile([128, GR], BF16, tag="G", name="G")

        # ---- 1. XT granules (+ value-source transposed for h) ----
        XT = []
        for di in range(nd):
            t = g_tile()
            if h_mode:
                nc.sync.dma_start_transpose(t[:], src_T[:, di * 128:(di + 1) * 128])
            else:
                nc.sync.dma_start(t[:], src_T[di])
            XT.append(t)
        if h_mode:
            VT_src = []
            for di in range(nv):
                t = g_tile()
                nc.sync.dma_start_transpose(t[:], vsrc_dram[:, di * 128:(di + 1) * 128])
                VT_src.append(t)
        else:
            VT_src = XT

        # ---- 2. QKT (transposed linear) ----
        QKT = []
        with tc.tile_pool(name=f"{n}_wk", bufs=2) as wp, \
             tc.tile_pool(name=f"{n}_pk", bufs=4, space="PSUM") as pp:
            for do in range(nd):
                w = wp.tile([128, d], BF16, tag="wst")
                nc.sync.dma_start(w[:], io[f"{n}_Wk_st"][do])
                bt = wp.tile([128, 1], F32, tag="bst")
                nc.sync.dma_start(bt[:], io[f"{n}_bk"][do])
                o = g_tile()
                for lc in range(L // 512):
                    ps = pp.tile([128, 512], F32, tag="ps")
                    for di in range(nd):
                        nc.tensor.matmul(ps[:], w[:, di * 128:(di + 1) * 128],
                                         XT[di][:, lc * 512:(lc + 1) * 512],
                                         start=(di == 0), stop=(di == nd - 1))
                    nc.scalar.activation(o[:, lc * 512:(lc + 1) * 512], ps[:],
                                         AF.Identity, bias=bt[:], scale=1.0)
                QKT.append(o)

        # ---- 3. V natural: V[lt] = list of ngr_v granules [128, GR] ----
        V = [[g_tile() for _ in range(ngr_v)] for _ in range(nlt)]
        with tc.tile_pool(name=f"{n}_wv", bufs=nv + 2) as wp, \
             tc.tile_pool(name=f"{n}_pv", bufs=4, space="PSUM") as pp:
            brow = wp.tile([1, vlen], BF16, tag="bvrow", bufs=1)
            nc.sync.dma_start(brow[:], io[f"{n}_bv_row"][:])
            for vc in range(vlen // 512):
                wt = []
                for di in range(nv):
                    w = wp.tile([128, 512], BF16, tag="wmv", name="wmv")
                    nc.sync.dma_start(w[:], io[f"{n}_Wv_mv"][di][:, vc * 512:(vc + 1) * 512])
                    wt.append(w)
                for lt in range(nlt):
                    ps = pp.tile([128, 512], F32, tag="ps")
                    for di in range(nv):
                        nc.tensor.matmul(ps[:], VT_src[di][:, lt * 128:(lt + 1) * 128],
                                         wt[di][:], start=(di == 0), stop=False)
                    nc.tensor.matmul(ps[:], C["ones_row"][:],
                                     brow[:, vc * 512:(vc + 1) * 512],
                                     start=False, stop=True)
                    g, col = (vc * 512) // GR, (vc * 512) % GR
                    nc.scalar.activation(V[lt][g][:, col:col + 512], ps[:], AF.Copy)

        # ---- 4. attention ----
        outT = [g_tile() for _ in range(nv)]
        ndk = dk // 128
        with tc.tile_pool(name=f"{n}_ws", bufs=2) as ws, \
             tc.tile_pool(name=f"{n}_wsT", bufs=12) as wsT, \
             tc.tile_pool(name=f"{n}_pa", bufs=2, space="PSUM") as pa, \
             tc.tile_pool(name=f"{n}_pt", bufs=2, space="PSUM") as pt, \
             tc.tile_pool(name=f"{n}_po", bufs=2, space="PSUM") as po:
            for h in range(H):
                qh = [QKT[(h * dk) // 128 + i] for i in range(ndk)]
                gam = C["gammas"][:, blk.idx * H + h:blk.idx * H + h + 1]
                for t in range(nlt):
                    Lk = (t + 1) * 128
                    ps_s = pa.tile([128, 1024], F32, tag="ps_s")
                    for kc in range((Lk + 511) // 512):
                        nk = min(512, Lk - kc * 512)
                        for ki in range(ndk):
                            nc.tensor.matmul(
                                ps_s[:, kc * 512:kc * 512 + nk],
                                qh[ki][:, t * 128:(t + 1) * 128],
                                qh[ki][:, kc * 512:kc * 512 + nk],
                                start=(ki == 0), stop=(ki == ndk - 1))
                    nc.vector.tensor_add(ps_s[:, t * 128:Lk], ps_s[:, t * 128:Lk], mask[:])
                    p = ws.tile([128, L], F32, tag="p")
                    nc.scalar.activation(p[:, :Lk], ps_s[:, :Lk], AF.Exp, scale=inv_s)
                    cum = ws.tile([128, L], F32, tag="cum")
                    nc.vector.tensor_tensor_scan(cum[:, :Lk], p[:, :Lk],
                                                 C["zeros"][:, :Lk], 0.0,
                                                 op0=OP.add, op1=OP.add)
                    sig = cum[:, Lk - 1:Lk]
                    rcp = ws.tile([128, 1], F32, tag="rcp")
                    nc.vector.tensor_scalar_add(rcp[:], sig, 1e-30)
                    nc.vector.reciprocal(rcp[:], rcp[:])
                    d2 = ws.tile([128, L], F32, tag="d2")
                    np_sl = C["negpos"][:, 1024 - 128 * t:1024 - 128 * t + Lk]
                    nc.vector.scalar_tensor_tensor(d2[:, :Lk], cum[:, :Lk], sig,
                                                   np_sl, op0=OP.subtract, op1=OP.mult)
                    nc.scalar.activation(d2[:, :Lk], d2[:, :Lk], AF.Ln, scale=rcp[:])
                    nc.scalar.activation(d2[:, :Lk], d2[:, :Lk], AF.Exp, scale=0.5)
                    nc.scalar.activation(d2[:, :Lk], d2[:, :Lk], AF.Exp, scale=gam)
                    wv = ws.tile([128, L], F32, tag="wv")
                    nc.vector.scalar_tensor_tensor(wv[:, :Lk], d2[:, :Lk], 1e-5,
                                                   ps_s[:, :Lk], op0=OP.max, op1=OP.mult)
                    au = ws.tile([128, L], F32, tag="au")
                    sig2 = ws.tile([128, 1], F32, tag="sig2")
                    nc.scalar.activation(au[:, :Lk], wv[:, :Lk], AF.Exp, scale=inv_s,
                                         accum_out=sig2[:])
                    rcp2 = ws.tile([128, 1], F32, tag="rcp2")
                    nc.vector.tensor_scalar_add(rcp2[:], sig2[:], 1e-30)
                    nc.vector.reciprocal(rcp2[:], rcp2[:])
                    abf = ws.tile([128, L], BF16, tag="abf")
                    nc.vector.tensor_scalar_mul(abf[:, :Lk], au[:, :Lk], rcp2[:])
                    aT = []
                    for s in range(t + 1):
                        pst = pt.tile([128, 128], BF16, tag="ps_tr")
                        nc.tensor.transpose(pst[:], abf[:, s * 128:(s + 1) * 128],
                                            C["identity"][:])
                        a = wsT.tile([128, 128], BF16, tag="aT")
                        nc.vector.tensor_copy(a[:], pst[:])
                        aT.append(a)
                    for dvi in range(dv // 128):
                        c = h * dv + dvi * 128
                        g, col = c // GR, c % GR
                        ps_o = po.tile([128, 128], F32, tag="ps_o")
                        for s in range(t + 1):
                            nc.tensor.matmul(ps_o[:], V[s][g][:, col:col + 128],
                                             aT[s][:], start=(s == 0), stop=(s == t))
                        ot = outT[c // 128]
                        nc.vector.tensor_copy(ot[:, t * 128:(t + 1) * 128], ps_o[:])

        # ---- 5. a2 natural + residual -> R granules; LN1 -> QLN ----
        R = [[None] * ngr_d for _ in range(nlt)]
        with tc.tile_pool(name=f"{n}_wo", bufs=2 * nv + 2) as wp, \
             tc.tile_pool(name=f"{n}_res", bufs=3) as rp, \
             tc.tile_pool(name=f"{n}_po2", bufs=4, space="PSUM") as pp:
            brow = wp.tile([1, d], BF16, tag="borow", bufs=1)
            nc.sync.dma_start(brow[:], io[f"{n}_bo_row"][:])
            for lt in range(nlt):
                for g in range(ngr_d):
                    R[lt][g] = g_tile()
            for dc in range(d // 512):
                wt = []
                for di in range(nv):
                    w = wp.tile([128, 512], BF16, tag="womv")
                    nc.sync.dma_start(w[:], io[f"{n}_Wo_mv"][di][:, dc * 512:(dc + 1) * 512])
                    wt.append(w)
                for lt in range(nlt):
                    ps = pp.tile([128, 512], F32, tag="ps")
                    for di in range(nv):
                        nc.tensor.matmul(ps[:], outT[di][:, lt * 128:(lt + 1) * 128],
                                         wt[di][:], start=(di == 0), stop=False)
                    nc.tensor.matmul(ps[:], C["ones_row"][:],
                                     brow[:, dc * 512:(dc + 1) * 512],
                                     start=False, stop=True)
                    xn = rp.tile([128, 512], BF16, tag="xn")
                    if h_mode:
                        nc.sync.dma_start(
                            xn[:], src_n[lt * 128:(lt + 1) * 128, dc * 512:(dc + 1) * 512])
                    else:
                        nc.sync.dma_start(xn[:], src_n[lt][:, dc * 512:(dc + 1) * 512])
                    g, col = (dc * 512) // GR, (dc * 512) % GR
                    nc.vector.tensor_add(R[lt][g][:, col:col + 512], ps[:], xn[:])

        QLN = [[None] * ngr_d for _ in range(nlt)]
        with tc.tile_pool(name=f"{n}_ln", bufs=3) as lp:
            for lt in range(nlt):
                for g in range(ngr_d):
                    QLN[lt][g] = g_tile()
                layernorm_grans(nc, lp, [R[lt][g][:] for g in range(ngr_d)], GR,
                                [QLN[lt][g][:] for g in range(ngr_d)])

        if not blk.ffn:
            for lt in range(nlt):
                for g in range(ngr_d):
                    nc.sync.dma_start(
                        out_n_dram[lt * 128:(lt + 1) * 128, g * GR:(g + 1) * GR],
                        QLN[lt][g][:])
            return

        # ---- 6. FFN ----
        for lt in range(nlt):
            for g in range(ngr_d):
                nc.sync.dma_start(
                    qln_dram[lt * 128:(lt + 1) * 128, g * GR:(g + 1) * GR],
                    QLN[lt][g][:])
        QLT = []
        for di in range(nd):
            t = g_tile()
            nc.sync.dma_start_transpose(t[:], qln_dram[:, di * 128:(di + 1) * 128])
            QLT.append(t)

        F1 = []
        with tc.tile_pool(name=f"{n}_w1", bufs=2) as wp, \
             tc.tile_pool(name=f"{n}_p1", bufs=4, space="PSUM") as pp:
            for do in range(DFF // 128):
                w = wp.tile([128, d], BF16, tag="w1st")
                nc.sync.dma_start(w[:], io[f"{n}_W1_st"][do])
                bt = wp.tile([128, 1], F32, tag="b1st")
                nc.sync.dma_start(bt[:], io[f"{n}_b1"][do])
                o = g_tile()
                for lc in range(L // 512):
                    ps = pp.tile([128, 512], F32, tag="ps")
                    for di in range(nd):
                        nc.tensor.matmul(ps[:], w[:, di * 128:(di + 1) * 128],
                                         QLT[di][:, lc * 512:(lc + 1) * 512],
                                         start=(di == 0), stop=(di == nd - 1))
                    nc.scalar.activation(o[:, lc * 512:(lc + 1) * 512], ps[:],
                                         AF.Relu, bias=bt[:], scale=1.0)
                F1.append(o)

        # FFN2 natural, residual accumulated in place into QLN, then LN2
        with tc.tile_pool(name=f"{n}_w2", bufs=DFF // 128 + 4) as wp, \
             tc.tile_pool(name=f"{n}_ln2", bufs=3) as lp, \
             tc.tile_pool(name=f"{n}_p2", bufs=4, space="PSUM") as pp:
            brow = wp.tile([1, d], BF16, tag="b2row", bufs=1)
            nc.sync.dma_start(brow[:], io[f"{n}_b2_row"][:])
            for dc in range(d // 512):
                wt = []
                for di in range(DFF // 128):
                    w = wp.tile([128, 512], BF16, tag="w2mv")
                    nc.sync.dma_start(w[:], io[f"{n}_W2_mv"][di][:, dc * 512:(dc + 1) * 512])
                    wt.append(w)
                for lt in range(nlt):
                    ps = pp.tile([128, 512], F32, tag="ps")
                    for di in range(DFF // 128):
                        nc.tensor.matmul(ps[:], F1[di][:, lt * 128:(lt + 1) * 128],
                                         wt[di][:], start=(di == 0), stop=False)
                    nc.tensor.matmul(ps[:], C["ones_row"][:],
                                     brow[:, dc * 512:(dc + 1) * 512],
                                     start=False, stop=True)
                    g, col = (dc * 512) // GR, (dc * 512) % GR
                    # in-place residual: QLN <- f2 + QLN
                    nc.vector.tensor_add(QLN[lt][g][:, col:col + 512], ps[:],
                                         QLN[lt][g][:, col:col + 512])
            for lt in range(nlt):
                if out_io is not None:  # final block: f32 natural straight out
                    o = lp.tile([128, d], F32, tag="ofin")
                    layernorm_grans(nc, lp, [QLN[lt][g][:] for g in range(ngr_d)], GR,
                                    [o[:, g * GR:(g + 1) * GR] for g in range(ngr_d)])
                    nc.sync.dma_start(out_io[lt], o[:])
                else:
                    obf = [g_tile() for _ in range(ngr_d)]
                    layernorm_grans(nc, lp, [QLN[lt][g][:] for g in range(ngr_d)], GR,
                                    [obf[g][:] for g in range(ngr_d)])
                    for g in range(ngr_d):
                        nc.sync.dma_start(
                            out_n_dram[lt * 128:(lt + 1) * 128, g * GR:(g + 1) * GR],
                            obf[g][:])


# ================================================================ runner

_CACHE = {}


def _get_compiled():
    if "nc" not in _CACHE:
        nc, io = build_kernel()
        nc.finalize()
        _CACHE["nc"] = nc
        _CACHE["io"] = io
    return _CACHE["nc"], _CACHE["io"]


def _softplus(x):
    return np.logaddexp(0.0, x)


def prep_in_maps(q_embed_data, qa_embed_data, params):
    q = np.asarray(q_embed_data, np.float32)
    qa = np.asarray(qa_embed_data, np.float32)

    shared = {}
    p = np.arange(128)[:, None]
    u = np.arange(2048)[None, :]
    shared["negpos"] = (-np.abs(p + 1024.0 - u)).astype(np.float32)
    i_ = np.arange(128)[:, None]
    j_ = np.arange(128)[None, :]
    shared["mask_incl"] = np.where(j_ <= i_, 0.0, NEG_MASK).astype(np.float32)
    shared["mask_strict"] = np.where(j_ < i_, 0.0, NEG_MASK).astype(np.float32)
    shared["identity"] = np.eye(128, dtype=NPBF)
    shared["ones_row"] = np.ones((1, 128), dtype=NPBF)
    gam = np.zeros((128, 3 * H), np.float32)
    for nm, bidx in [("y", 0), ("x", 1), ("h", 2)]:
        gvals = -_softplus(np.asarray(params[nm]["gam"], np.float32).reshape(H))
        gam[:, bidx * H:(bidx + 1) * H] = gvals[None, :]
    shared["gammas"] = gam

    for nm in ["y", "x", "h"]:
        bp = params[nm]
        shared[f"{nm}_Wk_st"] = prep_stationary(np.asarray(bp["Wk"], np.float32))
        shared[f"{nm}_bk"] = prep_bias_col(bp["bk"])
        shared[f"{nm}_Wv_mv"] = prep_moving(np.asarray(bp["Wv"], np.float32))
        shared[f"{nm}_bv_row"] = prep_bias_row(bp["bv"])
        shared[f"{nm}_Wo_mv"] = prep_moving(np.asarray(bp["Wo"], np.float32))
        shared[f"{nm}_bo_row"] = prep_bias_row(bp["bo"])
        if "W1" in bp:
            shared[f"{nm}_W1_st"] = prep_stationary(np.asarray(bp["W1"], np.float32))
            shared[f"{nm}_b1"] = prep_bias_col(bp["b1"])
            shared[f"{nm}_W2_mv"] = prep_moving(np.asarray(bp["W2"], np.float32))
            shared[f"{nm}_b2_row"] = prep_bias_row(bp["b2"])

    in_maps = []
    for b in range(B):
        m = dict(shared)
        m["qaT"] = np.ascontiguousarray(qa[b].T).reshape(16, 128, L).astype(NPBF)
        m["qa_n"] = np.ascontiguousarray(qa[b]).reshape(8, 128, 2 * D).astype(NPBF)
        m["qT"] = np.ascontiguousarray(q[b].T).reshape(8, 128, L).astype(NPBF)
        m["q_n"] = np.ascontiguousarray(q[b]).reshape(8, 128, D).astype(NPBF)
        in_maps.append(m)
    return in_maps


def kernel(q_embed_data, qa_embed_data, params):
    nc, io = _get_compiled()
    in_maps = prep_in_maps(q_embed_data, qa_embed_data, params)
    res = bass_utils.run_bass_kernel_spmd(nc, in_maps, core_ids=list(range(B)))
    outs = [r["out"].reshape(L, D) for r in res.results]
    return np.stack(outs, 0).astype(np.float32)
